# revision 16
# baseline (speedup 1.0000x reference)
"""Contrastive loss (SupCon-style) on 8 Trainium2 NeuronCores.

Reference (N=8192, D=1024, T=0.1):
    sim = emb @ emb.T / T;  e = exp(sim)
    all_sum_i = sum_j e_ij - e_ii
    pos_sum_i = sum_j e_ij * lab_j - e_ii * lab_i
    loss = mean_{i: lab_i=1} [ log(all_sum_i + eps) - log(pos_sum_i) ]
    (0.0 if n_ref < 2)

Sharding: rows are split across 8 cores (1024 rows each). Each core gets a
column-ROTATED copy of emb^T (rotated by its row offset) so that its diagonal
block lands at fixed local columns [0, 1024) -- this keeps the instruction
stream identical across cores (single SPMD NEFF) while row sums are invariant
under the column permutation.

Per core: sim tile [128 i x 512 j] = 8 accumulating fp32r matmuls over d;
diagonal positions get -BIG added pre-exp (exact self-exclusion); ScalarE
exp(scale*x) with accum_out emits all_sum for free; VectorE
tensor_tensor_reduce(exp * lab) emits pos_sum. Final log/mask/reduce on
device down to one partial scalar per core; host sums 8 partials / n_ref.
"""

import numpy as np

import concourse.bass as bass
import concourse.tile as tile
import concourse.mybir as mybir
from concourse import bacc
from concourse.bass_utils import run_bass_kernel_spmd

N, D = 8192, 1024
NCORES = 8
ROWS = N // NCORES  # 1024 rows per core
P = 128             # partitions
JS = 512            # j-slice width (one PSUM bank of fp32)
NJ = N // JS        # 16 j slices
ND = D // P         # 8 contraction chunks
IC = ROWS // P      # 8 row chunks per core
SCALE = 10.0        # 1 / TEMPERATURE
EPS = 1e-8
BIG = 1e9           # sim[diag] -= BIG before exp => exp -> 0

F32 = mybir.dt.float32
BF16 = mybir.dt.bfloat16
DT_MM = mybir.dt.float8e4

_build_cache = {}


def build(reps: int = 1, level: int = 3, dt_mm=None):
    """level: 0=mm+exp only, 1=+ttr, 2=+diag-sub, 3=full (fp32 ones-matmul).

    level<3 outputs per-row `contrib` [P, IC] (host sums); level>=3 outputs
    the scalar partial via the on-device ones-matmul partition reduction.
    """
    if dt_mm is None:
        dt_mm = DT_MM
    key = (reps, level, dt_mm)
    if key in _build_cache:
        return _build_cache[key]

    nc = bacc.Bacc("TRN2", target_bir_lowering=False, debug=False)
    embT_d = nc.dram_tensor("embT", [D, N], dt_mm, kind="ExternalInput")
    lab_d = nc.dram_tensor("lab", [N], BF16, kind="ExternalInput")
    labt_d = nc.dram_tensor("labt", [P, IC], F32, kind="ExternalInput")
    if level >= 3:
        partial_d = nc.dram_tensor("partial", [1, 1], F32, kind="ExternalOutput")
    else:
        partial_d = nc.dram_tensor("partial", [P, IC], F32, kind="ExternalOutput")

    # [D, N] viewed as [p, dc, n] with d = dc*128 + p
    embT = embT_d.ap().rearrange("(dc p) n -> p dc n", p=P)
    lab_bcast_src = bass.AP(tensor=lab_d, offset=0, ap=[[0, P], [1, N]])

    with tile.TileContext(nc) as tc:
        with (
            tc.tile_pool(name="consts", bufs=1) as consts,
            tc.tile_pool(name="rhsp", bufs=3) as rhsp,
            tc.tile_pool(name="expp", bufs=3) as expp,
            tc.tile_pool(name="scrp", bufs=2) as scrp,
            tc.tile_pool(name="stats", bufs=1) as stats,
            tc.tile_pool(name="fin", bufs=1) as fin,
            tc.tile_pool(name="psum", bufs=3, space=bass.MemorySpace.PSUM) as psum,
            tc.tile_pool(name="fpsum", bufs=1, space=bass.MemorySpace.PSUM) as fpsum,
        ):
            # resident stationary operand: this core's 1024 embedding columns
            res = consts.tile([P, ND, ROWS], dt_mm)
            nc.sync.dma_start(out=res, in_=embT[:, :, 0:ROWS])
            # labels broadcast across partitions [P, N] (bf16: 0/1 exact,
            # enables DVE 2x mode on the pos-sum pass)
            labb = consts.tile([P, N], BF16)
            nc.gpsimd.dma_start(out=labb, in_=lab_bcast_src)
            # per-row label mask in [p, ic] layout
            labt = consts.tile([P, IC], F32)
            nc.sync.dma_start(out=labt, in_=labt_d.ap())
            # BIG * identity (subtracted on the diagonal block pre-exp)
            bigI = consts.tile([P, P], F32)
            nc.gpsimd.memset(bigI, 0.0)
            nc.gpsimd.affine_select(
                out=bigI,
                in_=bigI,
                compare_op=mybir.AluOpType.not_equal,
                fill=BIG,
                base=0,
                pattern=[[-1, P]],
                channel_multiplier=1,
            )
            ones = consts.tile([P, 1], F32)
            nc.vector.memset(ones, 1.0)
            epsb = consts.tile([P, 1], F32)
            nc.vector.memset(epsb, EPS)

            for rep in range(reps):
                alls = stats.tile([P, IC * (NJ // 2)], F32, tag="alls")
                poss = stats.tile([P, IC * (NJ // 2)], F32, tag="poss")

                use_dr = dt_mm in mybir.MATMUL_PERF_MODE_DTYPES
                for jp in range(NJ // 2):
                    rhs = rhsp.tile([P, ND, 2 * JS], dt_mm, tag="rhs")
                    nc.sync.dma_start(
                        out=rhs, in_=embT[:, :, jp * 2 * JS : (jp + 1) * 2 * JS]
                    )
                    for ic in range(IC):
                        # two matmul groups -> one 2-bank PSUM tile, so the
                        # exp and pos-sum passes run at [P, 1024] granularity
                        ps = psum.tile([P, 2, JS], F32, tag="ps")
                        for s in range(2):
                            rhs_s = rhs[:, :, s * JS : (s + 1) * JS]
                            if use_dr:
                                for dc2 in range(ND // 2):
                                    nc.tensor.matmul(
                                        ps[:, s, :],
                                        res[:, 2 * dc2 : 2 * dc2 + 2, ic * P : (ic + 1) * P],
                                        rhs_s[:, 2 * dc2 : 2 * dc2 + 2, :],
                                        start=(dc2 == 0),
                                        stop=(dc2 == ND // 2 - 1),
                                        perf_mode=mybir.MatmulPerfMode.DoubleRow,
                                    )
                            else:
                                for dc in range(ND):
                                    nc.tensor.matmul(
                                        ps[:, s, :],
                                        res[:, dc, ic * P : (ic + 1) * P],
                                        rhs_s[:, dc, :],
                                        start=(dc == 0),
                                        stop=(dc == ND - 1),
                                    )
                        # diagonal block of this core sits at local columns
                        # [ic*128, ic*128+128): pair 0, half ic//4, offset (ic*128)%512
                        if level >= 2 and jp == 0:
                            off = (ic * P) % JS
                            nc.vector.tensor_sub(
                                ps[:, ic // 4, off : off + P],
                                ps[:, ic // 4, off : off + P],
                                bigI,
                            )
                        ext = expp.tile([P, 2 * JS], BF16, tag="ext")
                        idx = ic * (NJ // 2) + jp
                        nc.scalar.activation(
                            out=ext,
                            in_=ps.rearrange("p s j -> p (s j)"),
                            func=mybir.ActivationFunctionType.Exp,
                            scale=SCALE,
                            accum_out=alls[:, idx : idx + 1],
                        )
                        if level < 1:
                            continue
                        junk = scrp.tile([P, 2 * JS], BF16, tag="junk")
                        nc.vector.scalar_tensor_tensor(
                            out=junk,
                            in0=ext,
                            scalar=1.0,
                            in1=labb[:, jp * 2 * JS : (jp + 1) * 2 * JS],
                            op0=mybir.AluOpType.mult,
                            op1=mybir.AluOpType.mult,
                            accum_out=poss[:, idx : idx + 1],
                        )

                # ---- per-row loss and partial reduction ----
                asum = fin.tile([P, IC], F32, tag="asum")
                nc.vector.reduce_sum(
                    asum,
                    alls.rearrange("p (ic nj) -> p ic nj", nj=NJ // 2),
                    axis=mybir.AxisListType.X,
                )
                if level < 1:
                    nc.sync.dma_start(out=partial_d.ap(), in_=asum)
                    continue
                psumr = fin.tile([P, IC], F32, tag="psumr")
                nc.vector.reduce_sum(
                    psumr,
                    poss.rearrange("p (ic nj) -> p ic nj", nj=NJ // 2),
                    axis=mybir.AxisListType.X,
                )
                lnall = fin.tile([P, IC], F32, tag="lnall")
                nc.scalar.activation(
                    out=lnall,
                    in_=asum,
                    func=mybir.ActivationFunctionType.Ln,
                    bias=epsb,
                )
                lnpos = fin.tile([P, IC], F32, tag="lnpos")
                nc.scalar.activation(
                    out=lnpos,
                    in_=psumr,
                    func=mybir.ActivationFunctionType.Ln,
                )
                contrib = fin.tile([P, IC], F32, tag="contrib")
                nc.vector.tensor_sub(contrib, lnall, lnpos)
                nc.vector.tensor_mul(contrib, contrib, labt)
                if level >= 3:
                    # partition reduction via ones-matmul (fp32, tiny)
                    fps = fpsum.tile([1, IC], F32, tag="fps")
                    nc.tensor.matmul(fps, ones, contrib, start=True, stop=True)
                    stot = fin.tile([1, 1], F32, tag="stot")
                    nc.vector.reduce_sum(stot, fps, axis=mybir.AxisListType.X)
                    nc.sync.dma_start(out=partial_d.ap(), in_=stot)
                else:
                    nc.sync.dma_start(out=partial_d.ap(), in_=contrib)

    nc.compile()
    _build_cache[key] = nc
    return nc


def make_in_maps(embeddings: np.ndarray, labels: np.ndarray, dt_mm=None):
    if dt_mm is None:
        dt_mm = DT_MM
    emb = np.asarray(embeddings, dtype=np.float32)
    lab_f = np.asarray(labels).astype(np.float32)
    embT = np.ascontiguousarray(emb.T)  # [D, N]
    np_dt = mybir.dt.np(dt_mm)
    in_maps = []
    for c in range(NCORES):
        embT_rot = np.roll(embT, -c * ROWS, axis=1)
        lab_rot = np.roll(lab_f, -c * ROWS)
        labt = np.ascontiguousarray(
            lab_f[c * ROWS : (c + 1) * ROWS].reshape(IC, P).T
        )
        in_maps.append(
            {
                "embT": np.ascontiguousarray(embT_rot).astype(np_dt),
                "lab": lab_rot.astype(mybir.dt.np(BF16)),
                "labt": labt,
            }
        )
    return in_maps


def kernel(embeddings: np.ndarray, labels: np.ndarray) -> np.ndarray:
    lab_f = np.asarray(labels).astype(np.float32)
    n_ref = float(lab_f.sum())
    if n_ref < 2:
        return np.float32(0.0)

    nc = build(reps=1)
    in_maps = make_in_maps(embeddings, labels)
    res = run_bass_kernel_spmd(nc, in_maps, core_ids=list(range(NCORES)))
    total = np.float32(0.0)
    for c in range(NCORES):
        total += res.results[c]["partial"][0, 0]
    loss = total / np.float32(max(n_ref, 1.0))
    return np.asarray(loss, dtype=np.float32)
